# revision 3
# baseline (speedup 1.0000x reference)
"""Trainium2 Bass kernel for nn_DaVinciMLP (3-modality MoE MLP).

Reference computation (per token t with modality e = modality_ids[t]):
    xn  = bf16( x * rsqrt(mean(x^2) + 1e-6) * (norm_w[e] + 1) )
    up  = xn @ up_w[e].T            # [H] -> [I]
    g   = min(up, 7) * sigmoid(1.702 * min(up, 7))
    out = g @ down_w[e].T           # [I] -> [H]

Strategy:
  - Host: sort tokens by modality id so each expert's tokens are a dense,
    contiguous (128-padded) range -> dense per-expert GEMMs instead of the
    reference's 3x-masked-dense compute.  Fold (norm_w[e] + 1) into the up
    weights.
  - Sharding: Megatron tensor-parallel on the intermediate dim I across 8
    cores (up_w sharded on out dim, down_w on in dim).  Every core sees all
    tokens and produces a partial [H, L] output; host sums partials in f32.
  - Device: transposed activations [H, tok] are loaded straight from HBM
    with XBAR DMA-transpose; per 128-token tile the sum-of-squares runs on
    ScalarE (fused accum), rms = 1/sqrt(mean+eps) on VectorE, and the rms
    column is turned into a row with a tiny TensorE matmul, then broadcast
    across partitions by doubling SBUF-to-SBUF DMAs.  Up GEMM accumulates
    over H in PSUM; the rms scale is applied at the gelu stage
    (min(psum*rms,7)*sigmoid(1.702*...) on Vector+Scalar); down GEMM
    accumulates over the I-shard in PSUM and streams the partial output
    back transposed ([H, L]).
"""

import os
from contextlib import ExitStack

import numpy as np
import ml_dtypes

import concourse.bass as bass
import concourse.tile as tile
from concourse import bacc, mybir
from concourse.bass_utils import run_bass_kernel_spmd

BF16 = mybir.dt.bfloat16
F32 = mybir.dt.float32
NP_BF16 = ml_dtypes.bfloat16
AF = mybir.ActivationFunctionType

N_CORES = 8
H = 5120
I_FULL = 20480
E = 3
EPS = 1e-6
P = 128
TB = 1024  # max token block resident in SBUF
CHUNK = 512  # matmul moving free dim / PSUM bank width

LAST_EXEC_NS = None  # set when BASS_TRACE=1


def _build_program(blocks, L, h, i_shard, n_exp):
    """One SPMD program for all cores; per-core data differs only in values."""
    n_ko = h // P  # k-tiles over H for up GEMM; also # of H output blocks
    n_ic = i_shard // P  # I blocks per expert shard; k-tiles for down GEMM

    nc = bacc.Bacc()
    x_ext = nc.declare_dram_parameter("x", [L, h], BF16, isOutput=False)
    wup_ext = nc.declare_dram_parameter(
        "wup", [n_exp, n_ic, P, n_ko, P], BF16, isOutput=False
    )
    wd_ext = nc.declare_dram_parameter(
        "wd", [n_exp, n_ko, P, n_ic, P], BF16, isOutput=False
    )
    out_ext = nc.declare_dram_parameter("out", [h, L], BF16, isOutput=True)

    with tile.TileContext(nc) as tc, ExitStack() as ctx:
        const_pool = ctx.enter_context(tc.tile_pool(name="const", bufs=1))
        x_pool = ctx.enter_context(tc.tile_pool(name="x", bufs=2))
        sq_pool = ctx.enter_context(tc.tile_pool(name="sq", bufs=1))
        small_pool = ctx.enter_context(tc.tile_pool(name="small", bufs=4))
        rms_pool = ctx.enter_context(tc.tile_pool(name="rmsp", bufs=9))
        rbc_pool = ctx.enter_context(tc.tile_pool(name="rbc", bufs=2))
        rrow_pool = ctx.enter_context(tc.tile_pool(name="rrow", bufs=2))
        xT_pool = ctx.enter_context(tc.tile_pool(name="xT", bufs=1))
        g_pool = ctx.enter_context(tc.tile_pool(name="g", bufs=1))
        wu_pool = ctx.enter_context(tc.tile_pool(name="wu", bufs=3))
        wd_pool = ctx.enter_context(tc.tile_pool(name="wd", bufs=3))
        act_pool = ctx.enter_context(tc.tile_pool(name="act", bufs=2))
        ob_pool = ctx.enter_context(tc.tile_pool(name="ob", bufs=4))
        row_psum = ctx.enter_context(tc.tile_pool(name="rowps", bufs=1, space="PSUM"))
        up_psum = ctx.enter_context(tc.tile_pool(name="upps", bufs=3, space="PSUM"))
        dn_psum = ctx.enter_context(tc.tile_pool(name="dnps", bufs=2, space="PSUM"))
        bc_psum = ctx.enter_context(tc.tile_pool(name="bcps", bufs=2, space="PSUM"))

        from concourse.masks import make_identity

        ident_f = const_pool.tile([P, P], F32)
        make_identity(nc, ident_f)
        ones_bf = const_pool.tile([1, P], BF16)
        nc.vector.memset(ones_bf[:], 1.0)

        for (e, t0, ntok) in blocks:
            nt = (ntok + P - 1) // P
            xT = xT_pool.tile([P, n_ko, TB], BF16, tag="xT")
            gt = g_pool.tile([P, n_ic, TB], BF16, tag="g")
            rms_bc = rbc_pool.tile([P, TB], BF16, tag="rbc")

            # ---- transposed activation load (pure DMA via XBAR)
            for ko in range(n_ko):
                nc.sync.dma_start_transpose(
                    xT[:, ko, :ntok], x_ext[t0 : t0 + ntok, ko * P : (ko + 1) * P]
                )

            # prefetch the first up-weight blocks ahead of the x-stat loads
            wu_pref = {}
            for ic in range(min(3, n_ic)):
                wu = wu_pool.tile([P, n_ko, P], BF16, tag="wu")
                nc.scalar.dma_start(out=wu[:], in_=wup_ext[e, ic])
                wu_pref[ic] = wu

            # ---- per-token rms columns (ScalarE squares + small DVE chain);
            # the PE row-ification is deferred into the up phase so the PE
            # stream never stalls on this chain at a block boundary.
            rms_cols = []
            for t in range(nt):
                rt = min(P, ntok - t * P)
                xtile = x_pool.tile([P, h], BF16, tag="x")
                nc.sync.dma_start(
                    out=xtile[:rt], in_=x_ext[t0 + t * P : t0 + t * P + rt, :]
                )
                h2 = h // 2
                ssq2 = small_pool.tile([P, 2], F32, tag="ssq2")
                for half in range(2):
                    sq = sq_pool.tile([P, h2], BF16, tag="sq")
                    nc.scalar.activation(
                        sq[:rt],
                        xtile[:rt, half * h2 : (half + 1) * h2],
                        AF.Square,
                        accum_out=ssq2[:rt, half : half + 1],
                    )
                ssq = small_pool.tile([P, 1], F32, tag="ssq")
                nc.vector.tensor_tensor(
                    ssq[:rt], ssq2[:rt, 0:1], ssq2[:rt, 1:2], mybir.AluOpType.add
                )
                mn = small_pool.tile([P, 1], F32, tag="mn")
                nc.vector.tensor_scalar(
                    mn[:rt], ssq[:rt], 1.0 / h, EPS, mybir.AluOpType.mult, mybir.AluOpType.add
                )
                s_ = small_pool.tile([P, 1], F32, tag="s")
                nc.scalar.activation(s_[:rt], mn[:rt], AF.Sqrt)
                rms = rms_pool.tile([P, 1], F32, tag="rms")
                nc.vector.reciprocal(rms[:rt], s_[:rt])
                rms_cols.append((rt, rms))

            chunks = []
            c0 = 0
            while c0 < ntok:
                cw = min(CHUNK, ntok - c0)
                chunks.append((c0, cw))
                c0 += cw

            # ---- up GEMM + rms scale + gelu7 -> gt.  For ic==0 the rms
            # row-ify + partition-broadcast matmuls are slotted between the
            # chunk groups (stats are long done by then), and the gelu for
            # ic==0 is emitted only after the broadcast copies so the DVE
            # stream has no forward dependency on its own later entries.
            for ic in range(n_ic):
                if ic in wu_pref:
                    wu = wu_pref.pop(ic)
                else:
                    wu = wu_pool.tile([P, n_ko, P], BF16, tag="wu")
                    nc.scalar.dma_start(out=wu[:], in_=wup_ext[e, ic])
                ic0_ups = []
                rrow_tiles = []
                for (c0, cw) in chunks:
                    ups = up_psum.tile([P, CHUNK], F32, tag="upps")
                    for ko in range(n_ko):
                        nc.tensor.matmul(
                            ups[:, :cw],
                            lhsT=wu[:, ko, :],
                            rhs=xT[:, ko, c0 : c0 + cw],
                            start=(ko == 0),
                            stop=(ko == n_ko - 1),
                        )
                    if ic == 0:
                        # row-ify the rms columns covering this chunk
                        rr = row_psum.tile([1, CHUNK], F32, tag="rowps")
                        for t in range(c0 // P, (c0 + cw + P - 1) // P):
                            rt, rmst = rms_cols[t]
                            off = t * P - c0
                            nc.tensor.matmul(
                                rr[0:1, off : off + rt],
                                lhsT=rmst[:rt, 0:1],
                                rhs=ident_f[:rt, :rt],
                                start=True,
                                stop=True,
                            )
                        rrow = rrow_pool.tile([1, CHUNK], BF16, tag="rrow")
                        nc.vector.tensor_copy(out=rrow[0:1, :cw], in_=rr[0:1, :cw])
                        rrow_tiles.append(rrow)
                        ic0_ups.append(ups)
                    else:
                        tmin = act_pool.tile([P, CHUNK], BF16, tag="tmin")
                        nc.vector.tensor_tensor(
                            tmin[:, :cw],
                            ups[:, :cw],
                            rms_bc[:, c0 : c0 + cw],
                            mybir.AluOpType.mult,
                        )
                        nc.vector.tensor_scalar_min(tmin[:, :cw], tmin[:, :cw], 7.0)
                        sgm = act_pool.tile([P, CHUNK], BF16, tag="sgm")
                        nc.scalar.activation(
                            sgm[:, :cw], tmin[:, :cw], AF.Sigmoid, scale=1.702
                        )
                        nc.vector.tensor_mul(
                            out=gt[:, ic, c0 : c0 + cw], in0=tmin[:, :cw], in1=sgm[:, :cw]
                        )
                if ic == 0:
                    # partition-broadcast rms rows via K=1 ones-matmuls
                    for ci, (c0, cw) in enumerate(chunks):
                        bc_ps = bc_psum.tile([P, CHUNK], F32, tag="bcps")
                        nc.tensor.matmul(
                            bc_ps[:, :cw],
                            lhsT=ones_bf[0:1, :],
                            rhs=rrow_tiles[ci][0:1, :cw],
                            start=True,
                            stop=True,
                        )
                        nc.vector.tensor_copy(
                            out=rms_bc[:, c0 : c0 + cw], in_=bc_ps[:, :cw]
                        )
                    # deferred gelu for ic==0
                    for ci, (c0, cw) in enumerate(chunks):
                        ups = ic0_ups[ci]
                        tmin = act_pool.tile([P, CHUNK], BF16, tag="tmin")
                        nc.vector.tensor_tensor(
                            tmin[:, :cw],
                            ups[:, :cw],
                            rms_bc[:, c0 : c0 + cw],
                            mybir.AluOpType.mult,
                        )
                        nc.vector.tensor_scalar_min(tmin[:, :cw], tmin[:, :cw], 7.0)
                        sgm = act_pool.tile([P, CHUNK], BF16, tag="sgm")
                        nc.scalar.activation(
                            sgm[:, :cw], tmin[:, :cw], AF.Sigmoid, scale=1.702
                        )
                        nc.vector.tensor_mul(
                            out=gt[:, 0, c0 : c0 + cw], in0=tmin[:, :cw], in1=sgm[:, :cw]
                        )

            # ---- down GEMM -> partial out (transposed [H, L])
            for hc in range(n_ko):
                wdt = wd_pool.tile([P, n_ic, P], BF16, tag="wd")
                nc.scalar.dma_start(out=wdt[:], in_=wd_ext[e, hc])
                for (c0, cw) in chunks:
                    dps = dn_psum.tile([P, CHUNK], F32, tag="dnps")
                    for ko in range(n_ic):
                        nc.tensor.matmul(
                            dps[:, :cw],
                            lhsT=wdt[:, ko, :],
                            rhs=gt[:, ko, c0 : c0 + cw],
                            start=(ko == 0),
                            stop=(ko == n_ic - 1),
                        )
                    ob = ob_pool.tile([P, CHUNK], BF16, tag="ob")
                    nc.vector.tensor_copy(out=ob[:, :cw], in_=dps[:, :cw])
                    nc.sync.dma_start(
                        out=out_ext[hc * P : (hc + 1) * P, t0 + c0 : t0 + c0 + cw],
                        in_=ob[:, :cw],
                    )
    nc.compile()
    return nc


def _plan_blocks(ids, n_exp):
    """Sort tokens by expert, pad each segment to a multiple of 16 (XBAR row
    granularity), split into blocks of <= TB tokens (one expert per block)."""
    idx = [np.nonzero(ids == e)[0] for e in range(n_exp)]
    segs = []  # (expert, seg_start, n_valid)
    blocks = []  # (expert, tok_start, n_tok_padded)
    t0 = 0
    for e in range(n_exp):
        c = len(idx[e])
        if c == 0:
            continue
        cpad = ((c + 15) // 16) * 16
        off = 0
        while off < cpad:
            nb = min(TB, cpad - off)
            blocks.append((e, t0 + off, nb))
            off += nb
        segs.append((e, t0, c))
        t0 += cpad
    return idx, segs, blocks, t0


def _prep_weights(up_w, down_w, norm_w, h, i_full, n_exp, n_cores):
    """Fold (norm_w+1) into up weights; build per-core contiguous block
    layouts: wup [E, n_ic, ki, ko, m] (ki over H, m over I) and
    wd [E, n_hc, ki, ko, m] (ki over I, m over H)."""
    i_shard = i_full // n_cores
    n_ic = i_shard // P

    up = up_w.reshape(n_exp, i_full, h)
    dn = down_w.reshape(n_exp, h, i_full)
    w1 = norm_w.reshape(n_exp, 1, h).astype(np.float32) + 1.0

    # A[e, icg, ki, ko, m] = up[e, icg*P+m, ko*P+ki] * (norm_w[e, ko*P+ki]+1)
    A = np.empty((n_exp, i_full // P, P, h // P, P), dtype=NP_BF16)
    for e in range(n_exp):
        Ae = (up[e].astype(np.float32) * w1[e]).astype(NP_BF16)  # [I, H]
        A[e] = Ae.reshape(i_full // P, P, h // P, P).transpose(0, 3, 2, 1)
    # Bf[e, hc, ki, kog, m] = dn[e, hc*P+m, kog*P+ki]
    Bf = np.empty((n_exp, h // P, P, i_full // P, P), dtype=NP_BF16)
    for e in range(n_exp):
        Be = dn[e].astype(NP_BF16)  # [H, I]
        Bf[e] = Be.reshape(h // P, P, i_full // P, P).transpose(0, 3, 2, 1)

    wups, wds = [], []
    for c in range(n_cores):
        wups.append(np.ascontiguousarray(A[:, c * n_ic : (c + 1) * n_ic]))
        wds.append(np.ascontiguousarray(Bf[:, :, :, c * n_ic : (c + 1) * n_ic, :]))
    return wups, wds


def prepare_program(inputs):
    """Build the Bass program + per-core input maps (shared with bench.py)."""
    x = np.asarray(inputs["x"])
    ids = np.asarray(inputs["modality_ids"]).astype(np.int64)
    norm_w = np.asarray(inputs["norm_w"])
    up_w = np.asarray(inputs["up_w"])
    down_w = np.asarray(inputs["down_w"])

    n_tok, h = x.shape
    i_full = up_w.shape[0] // E
    assert down_w.shape == (E * h, i_full)
    if x.dtype != NP_BF16:
        x = x.astype(NP_BF16)

    idx, segs, blocks, L = _plan_blocks(ids, E)
    x_sorted = np.zeros((L, h), dtype=NP_BF16)
    for (e, s0, c) in segs:
        x_sorted[s0 : s0 + c] = x[idx[e]]

    wups, wds = _prep_weights(up_w, down_w, norm_w, h, i_full, E, N_CORES)

    nc = _build_program(blocks, L, h, i_full // N_CORES, E)
    in_maps = [{"x": x_sorted, "wup": wups[c], "wd": wds[c]} for c in range(N_CORES)]
    meta = {"idx": idx, "segs": segs, "L": L, "h": h, "n_tok": n_tok}
    return nc, in_maps, meta


def kernel(**inputs):
    global LAST_EXEC_NS
    # NTFF tracing needs axon hooks that aren't present in the sandbox; make
    # sure a stray BASS_TRACE can't divert run_bass_kernel_spmd into it.
    os.environ["BASS_NEVER_TRACE"] = "1"
    nc, in_maps, meta = prepare_program(inputs)
    idx, segs = meta["idx"], meta["segs"]
    n_tok, h, L = meta["n_tok"], meta["h"], meta["L"]
    res = run_bass_kernel_spmd(nc, in_maps, core_ids=list(range(N_CORES)))
    LAST_EXEC_NS = res.exec_time_ns

    acc = np.zeros((h, L), dtype=np.float32)
    for r in res.results:
        acc += np.asarray(r["out"], dtype=np.float32)
    out_sorted = acc.T  # [L, h]
    out = np.empty((n_tok, h), dtype=np.float32)
    for (e, s0, c) in segs:
        out[idx[e]] = out_sorted[s0 : s0 + c]
    return out.astype(NP_BF16)



# revision 4
# speedup vs baseline: 1.0231x; 1.0231x over previous
"""Trainium2 Bass kernel for nn_DaVinciMLP (3-modality MoE MLP).

Reference computation (per token t with modality e = modality_ids[t]):
    xn  = bf16( x * rsqrt(mean(x^2) + 1e-6) * (norm_w[e] + 1) )
    up  = xn @ up_w[e].T            # [H] -> [I]
    g   = min(up, 7) * sigmoid(1.702 * min(up, 7))
    out = g @ down_w[e].T           # [I] -> [H]

Strategy (v5):
  - Host: sort tokens by modality -> dense per-expert GEMMs; fold
    (norm_w+1) into up weights; Megatron TP over I across 8 cores; host
    sums the 8 partial [H, L] outputs.
  - Device: ALL input loads (XBAR transposes of x, row-major x for stats,
    up/down weights) ride the ACT HWDGE ring; output stores ride the SP
    ring alone.  HWDGE rings are in-order with per-entry waits, so stores
    (gated on compute) must not sit in front of the next block's loads.
  - The next block's loads (transposes, x, first up-weights) are emitted
    interleaved between the down-phase hc iterations so this block's wd
    loads are never queued behind a 24 MB prefetch burst.
  - Per-token rms: ScalarE square+accum -> small DVE chain -> per-128
    column tiles; the PE row-ify matmuls + K=1 ones-broadcast matmuls are
    slotted between up-phase chunk groups so the PE stream never waits on
    the stats chain at a block boundary.
"""

import os
from contextlib import ExitStack

import numpy as np
import ml_dtypes

import concourse.bass as bass
import concourse.tile as tile
from concourse import bacc, mybir
from concourse.bass_utils import run_bass_kernel_spmd

BF16 = mybir.dt.bfloat16
F32 = mybir.dt.float32
NP_BF16 = ml_dtypes.bfloat16
AF = mybir.ActivationFunctionType

N_CORES = 8
H = 5120
I_FULL = 20480
E = 3
EPS = 1e-6
P = 128
TB = 1024  # max token block resident in SBUF
CHUNK = 512  # matmul moving free dim / PSUM bank width

LAST_EXEC_NS = None


def _build_program(blocks, L, h, i_shard, n_exp):
    """One SPMD program for all cores; per-core data differs only in values."""
    n_ko = h // P
    n_ic = i_shard // P

    nc = bacc.Bacc()
    x_ext = nc.declare_dram_parameter("x", [L, h], BF16, isOutput=False)
    wup_ext = nc.declare_dram_parameter(
        "wup", [n_exp, n_ic, P, n_ko, P], BF16, isOutput=False
    )
    wd_ext = nc.declare_dram_parameter(
        "wd", [n_exp, n_ko, P, n_ic, P], BF16, isOutput=False
    )
    out_ext = nc.declare_dram_parameter("out", [h, L], BF16, isOutput=True)

    with tile.TileContext(nc) as tc, ExitStack() as ctx:
        const_pool = ctx.enter_context(tc.tile_pool(name="const", bufs=1))
        x_pool = ctx.enter_context(tc.tile_pool(name="x", bufs=2))
        sq_pool = ctx.enter_context(tc.tile_pool(name="sq", bufs=1))
        small_pool = ctx.enter_context(tc.tile_pool(name="small", bufs=4))
        rms_pool = ctx.enter_context(tc.tile_pool(name="rmsp", bufs=9))
        rbc_pool = ctx.enter_context(tc.tile_pool(name="rbc", bufs=2))
        rrow_pool = ctx.enter_context(tc.tile_pool(name="rrow", bufs=2))
        xT_pool = ctx.enter_context(tc.tile_pool(name="xT", bufs=1))
        g_pool = ctx.enter_context(tc.tile_pool(name="g", bufs=1))
        wu_pool = ctx.enter_context(tc.tile_pool(name="wu", bufs=3))
        wd_pool = ctx.enter_context(tc.tile_pool(name="wd", bufs=3))
        act_pool = ctx.enter_context(tc.tile_pool(name="act", bufs=2))
        ob_pool = ctx.enter_context(tc.tile_pool(name="ob", bufs=4))
        row_psum = ctx.enter_context(tc.tile_pool(name="rowps", bufs=1, space="PSUM"))
        up_psum = ctx.enter_context(tc.tile_pool(name="upps", bufs=3, space="PSUM"))
        dn_psum = ctx.enter_context(tc.tile_pool(name="dnps", bufs=2, space="PSUM"))
        bc_psum = ctx.enter_context(tc.tile_pool(name="bcps", bufs=2, space="PSUM"))

        from concourse.masks import make_identity

        ident_f = const_pool.tile([P, P], F32)
        make_identity(nc, ident_f)
        ones_bf = const_pool.tile([1, P], BF16)
        nc.vector.memset(ones_bf[:], 1.0)

        def emit_loads(e, t0, ntok, xT, x_tiles, wu_pref, first=False):
            """Return a list of thunks, each emitting one input-load DMA on
            the ACT ring, in priority order for the upcoming block."""
            thunks = []
            nt = (ntok + P - 1) // P

            def wu_load(ic):
                def f():
                    wu = wu_pool.tile([P, n_ko, P], BF16, tag="wu")
                    nc.scalar.dma_start(out=wu[:], in_=wup_ext[e, ic])
                    wu_pref[ic] = wu

                return f

            def tr_load(ko):
                def f():
                    nc.scalar.dma_start_transpose(
                        xT[:, ko, :ntok],
                        x_ext[t0 : t0 + ntok, ko * P : (ko + 1) * P],
                    )

                return f

            def x_load(t):
                def f():
                    rt = min(P, ntok - t * P)
                    xtile = x_pool.tile([P, h], BF16, tag="x")
                    nc.scalar.dma_start(
                        out=xtile[:rt], in_=x_ext[t0 + t * P : t0 + t * P + rt, :]
                    )
                    x_tiles.append(xtile)

                return f

            thunks.append(wu_load(0))
            if first:
                # startup: the first up matmul gates on the whole xT tile
                for ko in range(n_ko):
                    thunks.append(tr_load(ko))
                for t in range(nt):
                    thunks.append(x_load(t))
            else:
                for ko in range(n_ko // 2):
                    thunks.append(tr_load(ko))
                for t in range(min(4, nt)):
                    thunks.append(x_load(t))
                for ko in range(n_ko // 2, n_ko):
                    thunks.append(tr_load(ko))
                for t in range(4, nt):
                    thunks.append(x_load(t))
            for ic in range(1, min(3, n_ic)):
                thunks.append(wu_load(ic))
            return thunks

        # ---- first block's loads run up front
        e0, t00, ntok0 = blocks[0]
        xT_cur = xT_pool.tile([P, n_ko, TB], BF16, tag="xT")
        x_cur = []
        wu_pref = {}
        for th in emit_loads(e0, t00, ntok0, xT_cur, x_cur, wu_pref, first=True):
            th()

        for bi, (e, t0, ntok) in enumerate(blocks):
            nt = (ntok + P - 1) // P
            xT = xT_cur
            x_tiles = x_cur
            gt = g_pool.tile([P, n_ic, TB], BF16, tag="g")
            rms_bc = rbc_pool.tile([P, TB], BF16, tag="rbc")

            # ---- per-token rms columns (ScalarE squares + small DVE chain)
            rms_cols = []
            for t in range(nt):
                rt = min(P, ntok - t * P)
                xtile = x_tiles[t]
                h2 = h // 2
                ssq2 = small_pool.tile([P, 2], F32, tag="ssq2")
                for half in range(2):
                    sq = sq_pool.tile([P, h2], BF16, tag="sq")
                    nc.scalar.activation(
                        sq[:rt],
                        xtile[:rt, half * h2 : (half + 1) * h2],
                        AF.Square,
                        accum_out=ssq2[:rt, half : half + 1],
                    )
                ssq = small_pool.tile([P, 1], F32, tag="ssq")
                nc.vector.tensor_tensor(
                    ssq[:rt], ssq2[:rt, 0:1], ssq2[:rt, 1:2], mybir.AluOpType.add
                )
                mn = small_pool.tile([P, 1], F32, tag="mn")
                nc.vector.tensor_scalar(
                    mn[:rt], ssq[:rt], 1.0 / h, EPS,
                    mybir.AluOpType.mult, mybir.AluOpType.add,
                )
                s_ = small_pool.tile([P, 1], F32, tag="s")
                nc.scalar.activation(s_[:rt], mn[:rt], AF.Sqrt)
                rms = rms_pool.tile([P, 1], F32, tag="rms")
                nc.vector.reciprocal(rms[:rt], s_[:rt])
                rms_cols.append((rt, rms))

            chunks = []
            c0 = 0
            while c0 < ntok:
                cw = min(CHUNK, ntok - c0)
                chunks.append((c0, cw))
                c0 += cw

            # ---- up GEMM + rms scale + gelu7 -> gt
            wd_pref = {}
            for ic in range(n_ic):
                if ic in wu_pref:
                    wu = wu_pref.pop(ic)
                else:
                    wu = wu_pool.tile([P, n_ko, P], BF16, tag="wu")
                    nc.scalar.dma_start(out=wu[:], in_=wup_ext[e, ic])
                if ic == 2:
                    # early wd loads so the down phase doesn't queue behind
                    # the remaining wu loads on the ACT ring
                    for hcp in range(2):
                        wdt = wd_pool.tile([P, n_ic, P], BF16, tag="wd")
                        nc.scalar.dma_start(out=wdt[:], in_=wd_ext[e, hcp])
                        wd_pref[hcp] = wdt
                ic0_ups = []
                rrow_tiles = []
                for (c0, cw) in chunks:
                    ups = up_psum.tile([P, CHUNK], F32, tag="upps")
                    for ko in range(n_ko):
                        nc.tensor.matmul(
                            ups[:, :cw],
                            lhsT=wu[:, ko, :],
                            rhs=xT[:, ko, c0 : c0 + cw],
                            start=(ko == 0),
                            stop=(ko == n_ko - 1),
                        )
                    if ic == 0:
                        rr = row_psum.tile([1, CHUNK], F32, tag="rowps")
                        for t in range(c0 // P, (c0 + cw + P - 1) // P):
                            rt, rmst = rms_cols[t]
                            off = t * P - c0
                            nc.tensor.matmul(
                                rr[0:1, off : off + rt],
                                lhsT=rmst[:rt, 0:1],
                                rhs=ident_f[:rt, :rt],
                                start=True,
                                stop=True,
                            )
                        rrow = rrow_pool.tile([1, CHUNK], BF16, tag="rrow")
                        nc.vector.tensor_copy(out=rrow[0:1, :cw], in_=rr[0:1, :cw])
                        rrow_tiles.append(rrow)
                        ic0_ups.append(ups)
                    else:
                        tmin = act_pool.tile([P, CHUNK], BF16, tag="tmin")
                        nc.vector.tensor_tensor(
                            tmin[:, :cw],
                            ups[:, :cw],
                            rms_bc[:, c0 : c0 + cw],
                            mybir.AluOpType.mult,
                        )
                        nc.vector.tensor_scalar_min(tmin[:, :cw], tmin[:, :cw], 7.0)
                        sgm = act_pool.tile([P, CHUNK], BF16, tag="sgm")
                        nc.scalar.activation(
                            sgm[:, :cw], tmin[:, :cw], AF.Sigmoid, scale=1.702
                        )
                        nc.vector.tensor_mul(
                            out=gt[:, ic, c0 : c0 + cw], in0=tmin[:, :cw], in1=sgm[:, :cw]
                        )
                if ic == 0:
                    for ci, (c0, cw) in enumerate(chunks):
                        bc_ps = bc_psum.tile([P, CHUNK], F32, tag="bcps")
                        nc.tensor.matmul(
                            bc_ps[:, :cw],
                            lhsT=ones_bf[0:1, :],
                            rhs=rrow_tiles[ci][0:1, :cw],
                            start=True,
                            stop=True,
                        )
                        nc.vector.tensor_copy(
                            out=rms_bc[:, c0 : c0 + cw], in_=bc_ps[:, :cw]
                        )
                    for ci, (c0, cw) in enumerate(chunks):
                        ups = ic0_ups[ci]
                        tmin = act_pool.tile([P, CHUNK], BF16, tag="tmin")
                        nc.vector.tensor_tensor(
                            tmin[:, :cw],
                            ups[:, :cw],
                            rms_bc[:, c0 : c0 + cw],
                            mybir.AluOpType.mult,
                        )
                        nc.vector.tensor_scalar_min(tmin[:, :cw], tmin[:, :cw], 7.0)
                        sgm = act_pool.tile([P, CHUNK], BF16, tag="sgm")
                        nc.scalar.activation(
                            sgm[:, :cw], tmin[:, :cw], AF.Sigmoid, scale=1.702
                        )
                        nc.vector.tensor_mul(
                            out=gt[:, 0, c0 : c0 + cw], in0=tmin[:, :cw], in1=sgm[:, :cw]
                        )

            # ---- build the next block's load queue (emitted interleaved
            # into the down loop below)
            pro_q = []
            wu_next = {}
            if bi + 1 < len(blocks):
                e_n, t0_n, ntok_n = blocks[bi + 1]
                xT_cur = xT_pool.tile([P, n_ko, TB], BF16, tag="xT")
                x_cur = []
                pro_q = emit_loads(e_n, t0_n, ntok_n, xT_cur, x_cur, wu_next)

            # ---- down GEMM -> partial out (transposed [H, L]); out stores
            # ride the SP ring alone
            for hc in range(n_ko):
                if hc in wd_pref:
                    wdt = wd_pref.pop(hc)
                else:
                    wdt = wd_pool.tile([P, n_ic, P], BF16, tag="wd")
                    nc.scalar.dma_start(out=wdt[:], in_=wd_ext[e, hc])
                for (c0, cw) in chunks:
                    dps = dn_psum.tile([P, CHUNK], F32, tag="dnps")
                    for ko in range(n_ic):
                        nc.tensor.matmul(
                            dps[:, :cw],
                            lhsT=wdt[:, ko, :],
                            rhs=gt[:, ko, c0 : c0 + cw],
                            start=(ko == 0),
                            stop=(ko == n_ic - 1),
                        )
                    ob = ob_pool.tile([P, CHUNK], BF16, tag="ob")
                    nc.vector.tensor_copy(out=ob[:, :cw], in_=dps[:, :cw])
                    nc.sync.dma_start(
                        out=out_ext[hc * P : (hc + 1) * P, t0 + c0 : t0 + c0 + cw],
                        in_=ob[:, :cw],
                    )
                # drip the next block's loads between hc iterations
                if hc >= 1:
                    for _ in range(2):
                        if pro_q:
                            pro_q.pop(0)()
            while pro_q:
                pro_q.pop(0)()
            wu_pref = wu_next

    nc.compile()
    return nc


def _plan_blocks(ids, n_exp):
    """Sort tokens by expert, pad each segment to a multiple of 16 (XBAR row
    granularity), split into blocks of <= TB tokens (one expert per block)."""
    idx = [np.nonzero(ids == e)[0] for e in range(n_exp)]
    segs = []
    blocks = []
    t0 = 0
    for e in range(n_exp):
        c = len(idx[e])
        if c == 0:
            continue
        cpad = ((c + 15) // 16) * 16
        off = 0
        while off < cpad:
            nb = min(TB, cpad - off)
            blocks.append((e, t0 + off, nb))
            off += nb
        segs.append((e, t0, c))
        t0 += cpad
    return idx, segs, blocks, t0


def _prep_weights(up_w, down_w, norm_w, h, i_full, n_exp, n_cores):
    """Fold (norm_w+1) into up weights; build per-core contiguous block
    layouts: wup [E, n_ic, ki, ko, m] (ki over H, m over I) and
    wd [E, n_hc, ki, ko, m] (ki over I, m over H)."""
    i_shard = i_full // n_cores
    n_ic = i_shard // P

    up = up_w.reshape(n_exp, i_full, h)
    dn = down_w.reshape(n_exp, h, i_full)
    w1 = norm_w.reshape(n_exp, 1, h).astype(np.float32) + 1.0

    A = np.empty((n_exp, i_full // P, P, h // P, P), dtype=NP_BF16)
    for e in range(n_exp):
        Ae = (up[e].astype(np.float32) * w1[e]).astype(NP_BF16)
        A[e] = Ae.reshape(i_full // P, P, h // P, P).transpose(0, 3, 2, 1)
    Bf = np.empty((n_exp, h // P, P, i_full // P, P), dtype=NP_BF16)
    for e in range(n_exp):
        Be = dn[e].astype(NP_BF16)
        Bf[e] = Be.reshape(h // P, P, i_full // P, P).transpose(0, 3, 2, 1)

    wups, wds = [], []
    for c in range(n_cores):
        wups.append(np.ascontiguousarray(A[:, c * n_ic : (c + 1) * n_ic]))
        wds.append(np.ascontiguousarray(Bf[:, :, :, c * n_ic : (c + 1) * n_ic, :]))
    return wups, wds


def prepare_program(inputs):
    """Build the Bass program + per-core input maps (shared with bench.py)."""
    x = np.asarray(inputs["x"])
    ids = np.asarray(inputs["modality_ids"]).astype(np.int64)
    norm_w = np.asarray(inputs["norm_w"])
    up_w = np.asarray(inputs["up_w"])
    down_w = np.asarray(inputs["down_w"])

    n_tok, h = x.shape
    i_full = up_w.shape[0] // E
    assert down_w.shape == (E * h, i_full)
    if x.dtype != NP_BF16:
        x = x.astype(NP_BF16)

    idx, segs, blocks, L = _plan_blocks(ids, E)
    x_sorted = np.zeros((L, h), dtype=NP_BF16)
    for (e, s0, c) in segs:
        x_sorted[s0 : s0 + c] = x[idx[e]]

    wups, wds = _prep_weights(up_w, down_w, norm_w, h, i_full, E, N_CORES)

    nc = _build_program(blocks, L, h, i_full // N_CORES, E)
    in_maps = [{"x": x_sorted, "wup": wups[c], "wd": wds[c]} for c in range(N_CORES)]
    meta = {"idx": idx, "segs": segs, "L": L, "h": h, "n_tok": n_tok}
    return nc, in_maps, meta


def kernel(**inputs):
    global LAST_EXEC_NS
    # NTFF tracing needs axon hooks that aren't present in the sandbox; make
    # sure a stray BASS_TRACE can't divert run_bass_kernel_spmd into it.
    os.environ["BASS_NEVER_TRACE"] = "1"
    nc, in_maps, meta = prepare_program(inputs)
    idx, segs = meta["idx"], meta["segs"]
    n_tok, h, L = meta["n_tok"], meta["h"], meta["L"]
    res = run_bass_kernel_spmd(nc, in_maps, core_ids=list(range(N_CORES)))
    LAST_EXEC_NS = res.exec_time_ns

    acc = np.zeros((h, L), dtype=np.float32)
    for r in res.results:
        acc += np.asarray(r["out"], dtype=np.float32)
    out_sorted = acc.T  # [L, h]
    out = np.empty((n_tok, h), dtype=np.float32)
    for (e, s0, c) in segs:
        out[idx[e]] = out_sorted[s0 : s0 + c]
    return out.astype(NP_BF16)
